# revision 23
# baseline (speedup 1.0000x reference)
"""Trainium2 Bass kernel for nn_LogicNetwork (template-matching logic network).

Sharding: pure data parallel over 8 NeuronCores (batch split 8 ways).
Host-side prep per shard: XT = [state; state^2].T -> [36, n]; all rule
parameters fold into small constant matrices so per-element compute becomes
matmuls (TensorE), exps (ScalarE) and short reductions (VectorE/GpSimd):

    ms1 = M1.T @ XT (+C1)         E1 = exp(-ms1)            [108 = 12 rules x 9]
    S1 = SUM.T @ E1 ; T1 = SUM.T @ (E1*by) ; PS = max9(E1)  (via transpose to A)
    Xcf = [cfx, cfx^2, _, cfy, _, cfy^2] (cfx=PS, cfy=T1/S1) [76, n] (gapped rows)
    ms2 = M2.T @ Xcf (+C2)        E2 = exp(-ms2)            [144 = 12 x 12]
    S2,Tx,Ty = SUM2.T @ [E2, E2*cx, E2*cy]
    sel = [Tx;Ty]/S2              qT = Gx.T@selx + Gy.T@sely [9, n]
    host: out = qT.T + cvec

Layout B on-chip (feature on partitions, batch on free axis, 512-wide chunks),
float32r matmuls (1 cycle/row). Pattern-strength max runs in layout A after a
PE transpose of E1. PSUM banks are tag-recycled; ms1 bank is double-buffered
for cross-chunk overlap.
"""
import functools
import numpy as np

import concourse.bacc as bacc
import concourse.bass as bass
import concourse.tile as tile
from concourse import mybir
from concourse.bass_utils import run_bass_kernel_spmd

N_CORES = 8
TRACE = False
LAST_RESULT = None
B_TOTAL = 524288
B_CORE = B_TOTAL // N_CORES        # 65536
NB = 512                           # batch columns per chunk
F32 = mybir.dt.float32
F32R = mybir.dt.float32r
EXP = mybir.ActivationFunctionType.Exp
COPY = mybir.ActivationFunctionType.Copy

# Xcf row layout (gaps keep 32-alignment for engine writes; M2 rows at gaps = 0)
XCF_CFX = 0     # 0:12   cfx        (ScalarE copy, with cfx^2, from transpose)
XCF_CFX2 = 12   # 12:24  cfx^2
XCF_CFY = 32    # 32:44  cfy        (VectorE)
XCF_CFY2 = 64   # 64:76  cfy^2      (VectorE)
XCF_ROWS = 76


def _build_consts(c_templates, c_gammas, a_templates, a_gammas,
                  a_body_W, a_body_b, a_head_W, a_head_b, act_W, act_b):
    ct = c_templates.astype(np.float64)
    w1 = 1.0 - np.clip(c_gammas.astype(np.float64), 0.0, 1.0)
    C1 = (w1 * ct**2).sum(-1)
    A1 = -2.0 * w1 * ct
    M1 = np.zeros((36, 108), np.float32)
    SUM1 = np.zeros((108, 24), np.float32)   # cols 0:12 S1 (E1), 12:24 T1 (E1*by)
    for r in range(12):
        for i in range(9):
            col = r * 9 + i
            for p in range(2):
                M1[2 * i + p, col] = A1[r, p]
                M1[18 + 2 * i + p, col] = w1[r, p]
            SUM1[col, r] = 1.0
            SUM1[col, 12 + r] = 1.0
    bias1 = np.repeat(-C1, 9).astype(np.float32)[:, None]

    at = a_templates.reshape(12, 2).astype(np.float64)
    w2 = (1.0 - np.clip(a_gammas.astype(np.float64), 0.0, 1.0)).reshape(12, 2)
    C2 = (w2 * at**2).sum(-1)
    A2 = -2.0 * w2 * at
    M2 = np.zeros((XCF_ROWS, 144), np.float32)
    CXSEL = np.zeros((XCF_ROWS, 144), np.float32)
    CYSEL = np.zeros((XCF_ROWS, 144), np.float32)
    SUM2 = np.zeros((144, 36), np.float32)   # cols 0:12 S2, 12:24 Tx, 24:36 Ty
    for rj in range(12):
        for ii in range(12):
            col = rj * 12 + ii
            M2[XCF_CFX + ii, col] = A2[rj, 0]
            M2[XCF_CFX2 + ii, col] = w2[rj, 0]
            M2[XCF_CFY + ii, col] = A2[rj, 1]
            M2[XCF_CFY2 + ii, col] = w2[rj, 1]
            CXSEL[XCF_CFX + ii, col] = 1.0
            CYSEL[XCF_CFY + ii, col] = 1.0
            SUM2[col, rj] = 1.0
            SUM2[col, 12 + rj] = 1.0
            SUM2[col, 24 + rj] = 1.0
    bias2 = np.repeat(-C2, 12).astype(np.float32)[:, None]
    # merged b-half of cx/cy replication: cols 0:16 -> cx (ms2 cols 128:144),
    # cols 32:48 -> cy
    SELB2 = np.zeros((XCF_ROWS, 48), np.float32)
    SELB2[:, 0:16] = CXSEL[:, 128:144]
    SELB2[:, 32:48] = CYSEL[:, 128:144]

    bW = a_body_W.astype(np.float64)
    bb = a_body_b.astype(np.float64)
    hW = a_head_W.astype(np.float64)
    hb = a_head_b.astype(np.float64)
    aW = act_W.astype(np.float64)
    # Gx/Gy [12, 9]: row rj = r2*2+j; x uses l'=0, y uses l'=1
    G = np.zeros((2, 12, 9), np.float32)
    for r2 in range(6):
        for j in range(2):
            for lp in range(2):
                for a in range(9):
                    G[lp, r2 * 2 + j, a] = (
                        aW[a, r2 * 2:r2 * 2 + 2] @ (hW[r2] @ bW[r2, j, :, lp])
                    )
    cvec = aW @ (hb + (hW * bb.sum(1)[:, None, :]).sum(-1)).reshape(12)
    cvec = (cvec + act_b.astype(np.float64)).astype(np.float32)
    return dict(m1=M1, sum1=SUM1, bias1=bias1,
                m2a=np.ascontiguousarray(M2[:, 0:128]),
                m2b=np.ascontiguousarray(M2[:, 128:144]),
                cxsela=np.ascontiguousarray(CXSEL[:, 0:128]),
                cysela=np.ascontiguousarray(CYSEL[:, 0:128]),
                selb2=SELB2,
                sum2a=np.ascontiguousarray(SUM2[0:128]),
                sum2b=np.ascontiguousarray(SUM2[128:144]),
                bias2a=np.ascontiguousarray(bias2[0:128]),
                bias2b=np.ascontiguousarray(bias2[128:144]),
                gx=G[0], gy=G[1], ident=np.eye(128, dtype=np.float32),
                zeros=np.zeros((28, NB), np.float32)), cvec


CONST_SHAPES = dict(m1=(36, 108), sum1=(108, 24), bias1=(108, 1),
                    m2a=(XCF_ROWS, 128), m2b=(XCF_ROWS, 16),
                    cxsela=(XCF_ROWS, 128), cysela=(XCF_ROWS, 128),
                    selb2=(XCF_ROWS, 48),
                    sum2a=(128, 36), sum2b=(16, 36), bias2a=(128, 1),
                    bias2b=(16, 1), gx=(12, 9), gy=(12, 9), ident=(128, 128), zeros=(28, NB))
CONST_DTYPES = {name: (mybir.dt.float32 if name in ("bias1", "bias2a", "bias2b", "ident")
                       else mybir.dt.float32r) for name in CONST_SHAPES}


def _build_kernel(b_core=B_CORE):
    nchunk = b_core // NB
    nc = bacc.Bacc("TRN2", target_bir_lowering=False, debug=False,
                   enable_asserts=False, num_devices=N_CORES)
    xt_d = nc.dram_tensor("xt", [36, b_core], F32R, kind="ExternalInput").ap()
    const_d = {name: nc.dram_tensor(name, list(shp), CONST_DTYPES[name],
                                    kind="ExternalInput").ap()
               for name, shp in CONST_SHAPES.items()}
    qt_d = nc.dram_tensor("qt", [9, b_core], F32, kind="ExternalOutput").ap()

    with tile.TileContext(nc) as tc:
        with (tc.tile_pool(name="singles", bufs=1) as singles,
              tc.tile_pool(name="sb", bufs=6) as sb,
              tc.tile_pool(name="ps_p1", bufs=1, space="PSUM") as ps_p1,
              tc.tile_pool(name="ps_reds", bufs=1, space="PSUM") as ps_reds,
              tc.tile_pool(name="ps_p2a", bufs=1, space="PSUM") as ps_p2a,
              tc.tile_pool(name="ps_aux", bufs=1, space="PSUM") as ps_aux,
              tc.tile_pool(name="ps_auxb", bufs=1, space="PSUM") as ps_auxb,
              tc.tile_pool(name="ps_e1a", bufs=1, space="PSUM") as ps_e1a,
              tc.tile_pool(name="ps_tr", bufs=2, space="PSUM") as ps_tr):
            cs = {}
            for name, shp in CONST_SHAPES.items():
                cs[name] = singles.tile(list(shp), CONST_DTYPES[name],
                                        name=name, tag=name)
                nc.sync.dma_start(out=cs[name][:], in_=const_d[name])

            for k in range(nchunk):
                c0 = k * NB
                xt = sb.tile([36, NB], F32R, tag="xt")
                nc.sync.dma_start(out=xt[:], in_=xt_d[:, c0:c0 + NB])
                # by replicated to (r,i) rows straight from DRAM: row 2i+1
                by_sb = sb.tile([108, NB], F32R, tag="by_sb")
                by_src = bass.AP(tensor=xt_d.tensor, offset=b_core + c0,
                                 ap=[[0, 12], [2 * b_core, 9], [1, NB]])
                nc.sync.dma_start(out=by_sb[:], in_=by_src)

                # ---- stage 1 ----
                p1 = ps_p1.tile([108, NB], F32, tag="p1")
                nc.tensor.matmul(p1[:], cs["m1"][:], xt[:], start=True, stop=True)
                e1 = sb.tile([108, NB], F32R, tag="e1")
                nc.scalar.activation(e1[:], p1[:], EXP, bias=cs["bias1"][:],
                                     scale=-1.0)
                e1b = sb.tile([108, NB], F32R, tag="e1b")
                nc.gpsimd.tensor_mul(e1b[:], e1[:], by_sb[:])

                s1p = ps_reds.tile([12, NB], F32, tag="reds")
                nc.tensor.matmul(s1p[:], cs["sum1"][:, 0:12], e1[:],
                                 start=True, stop=True)
                t1p = ps_p2a.tile([12, NB], F32, tag="p2a")
                nc.tensor.matmul(t1p[:], cs["sum1"][:, 12:24], e1b[:],
                                 start=True, stop=True)

                # ---- PS = max9(E1) via layout A ----
                e1a = ps_e1a.tile([128, 4, 108], F32, tag="e1a")
                for j in range(4):
                    nc.tensor.transpose(e1a[:, j, :],
                                        e1[:, j * 128:(j + 1) * 128].bitcast(F32),
                                        cs["ident"][0:108, 0:108])
                psa = sb.tile([128, 4, 24], F32, tag="psa")
                nc.vector.tensor_reduce(
                    psa[:, :, 0:12],
                    e1a[:].rearrange("p f (r i) -> p (f r) i", r=12),
                    axis=mybir.AxisListType.X, op=mybir.AluOpType.max)
                nc.vector.tensor_mul(psa[:, :, 12:24], psa[:, :, 0:12],
                                     psa[:, :, 0:12])
                trp = ps_tr.tile([24, NB], F32, tag="tr")
                for j in range(4):
                    nc.tensor.transpose(trp[:, j * 128:(j + 1) * 128],
                                        psa[:, j, :], cs["ident"][:])

                # ---- Xcf assembly [76, NB] ----
                xcf = sb.tile([XCF_ROWS, NB], F32R, tag="xcf")
                if k < 3:  # slots cycle with bufs=3; zero gap rows once/slot
                    nc.sync.dma_start(out=xcf[24:32, :], in_=const_d["zeros"][0:8])
                    nc.sync.dma_start(out=xcf[44:64, :], in_=const_d["zeros"][8:28])
                nc.scalar.activation(xcf[0:24, :], trp[:], COPY)
                r1rec = sb.tile([12, NB], F32, tag="r1rec")
                nc.vector.reciprocal_approx_fast(out=r1rec[:], in_=s1p[:])
                nc.vector.tensor_mul(xcf[XCF_CFY:XCF_CFY + 12, :], t1p[:], r1rec[:])
                nc.vector.tensor_mul(xcf[XCF_CFY2:XCF_CFY2 + 12, :],
                                     xcf[XCF_CFY:XCF_CFY + 12, :],
                                     xcf[XCF_CFY:XCF_CFY + 12, :])

                # ---- stage 2 ----
                p2a = ps_p2a.tile([128, NB], F32, tag="p2a")
                nc.tensor.matmul(p2a[:], cs["m2a"][:], xcf[:], start=True, stop=True)
                p2b = ps_reds.tile([16, NB], F32, tag="reds")
                nc.tensor.matmul(p2b[:], cs["m2b"][:], xcf[:], start=True, stop=True)
                e2a = sb.tile([128, NB], F32R, tag="e2a")
                e2b = sb.tile([16, NB], F32R, tag="e2b")
                nc.scalar.activation(e2a[:], p2a[:], EXP, bias=cs["bias2a"][:],
                                     scale=-1.0)
                nc.scalar.activation(e2b[:], p2b[:], EXP, bias=cs["bias2b"][:],
                                     scale=-1.0)

                exy = sb.tile([128, 4, NB], F32R, tag="exy")  # exa|exb|eya|eyb
                repb = ps_auxb.tile([48, NB], F32, tag="auxb")
                nc.tensor.matmul(repb[:], cs["selb2"][:], xcf[:],
                                 start=True, stop=True)
                for lo, sel_name in ((0, "cxsela"), (2, "cysela")):
                    rep_a = ps_aux.tile([128, NB], F32, tag="aux")
                    nc.tensor.matmul(rep_a[:], cs[sel_name][:], xcf[:],
                                     start=True, stop=True)
                    nc.vector.tensor_mul(exy[0:128, lo, :], e2a[:], rep_a[:])
                    nc.vector.tensor_mul(exy[0:16, lo + 1, :], e2b[:],
                                         repb[lo * 16:lo * 16 + 16, :])

                s2p = ps_aux.tile([12, NB], F32, tag="aux")
                txp = ps_reds.tile([12, NB], F32, tag="reds")
                typ = ps_p2a.tile([12, NB], F32, tag="p2a")
                for dst, col, src_a, src_b in (
                        (s2p, 0, e2a[:], e2b[:]),
                        (txp, 12, exy[0:128, 0, :], exy[0:16, 1, :]),
                        (typ, 24, exy[0:128, 2, :], exy[0:16, 3, :])):
                    nc.tensor.matmul(dst[:], cs["sum2a"][:, col:col + 12],
                                     src_a, start=True, stop=False)
                    nc.tensor.matmul(dst[:], cs["sum2b"][:, col:col + 12],
                                     src_b, start=False, stop=True)

                # sel = [Tx;Ty]/S2 ; qT = Gx.T @ selx + Gy.T @ sely
                r2rec = sb.tile([12, NB], F32, tag="r2rec")
                nc.vector.reciprocal_approx_fast(out=r2rec[:], in_=s2p[:])
                selx = sb.tile([12, NB], F32R, tag="selx")
                sely = sb.tile([12, NB], F32R, tag="sely")
                nc.vector.tensor_mul(selx[:], txp[:], r2rec[:])
                nc.vector.tensor_mul(sely[:], typ[:], r2rec[:])

                qp = ps_tr.tile([9, NB], F32, tag="tr")
                nc.tensor.matmul(qp[:], cs["gx"][:], selx[:], start=True, stop=False)
                nc.tensor.matmul(qp[:], cs["gy"][:], sely[:], start=False, stop=True)
                q_sb = sb.tile([9, NB], F32, tag="q_sb")
                nc.scalar.activation(q_sb[:], qp[:], COPY)
                nc.sync.dma_start(out=qt_d[:, c0:c0 + NB], in_=q_sb[:])

    nc.compile()
    return nc


@functools.lru_cache(maxsize=2)
def _get_compiled(b_core=B_CORE):
    return _build_kernel(b_core)


def timeline_estimate(b_core=B_CORE):
    """Per-core kernel time estimate (ns) from the instruction cost model."""
    from concourse.timeline_sim import TimelineSim
    nc = _get_compiled(b_core)
    ts = TimelineSim(nc)
    ts.simulate()
    return ts


def kernel(state, c_templates, c_gammas, a_templates, a_gammas,
           a_body_W, a_body_b, a_head_W, a_head_b, act_W, act_b):
    state = np.asarray(state, np.float32)
    consts, cvec = _build_consts(
        np.asarray(c_templates, np.float32), np.asarray(c_gammas, np.float32),
        np.asarray(a_templates, np.float32), np.asarray(a_gammas, np.float32),
        np.asarray(a_body_W, np.float32), np.asarray(a_body_b, np.float32),
        np.asarray(a_head_W, np.float32), np.asarray(a_head_b, np.float32),
        np.asarray(act_W, np.float32), np.asarray(act_b, np.float32))

    b = state.shape[0]
    b_core = b // N_CORES
    nc = _get_compiled(b_core)
    in_maps = []
    for c in range(N_CORES):
        shard = state[c * b_core:(c + 1) * b_core]
        xt = np.ascontiguousarray(
            np.concatenate([shard, shard * shard], axis=1).T)
        in_maps.append({"xt": xt, **consts})
    global LAST_RESULT
    res = run_bass_kernel_spmd(nc, in_maps, core_ids=list(range(N_CORES)))
    LAST_RESULT = res
    out = np.empty((b, 9), np.float32)
    for c in range(N_CORES):
        out[c * b_core:(c + 1) * b_core] = res.results[c]["qt"].T
    out += cvec[None, :]
    return out
